# revision 22
# baseline (speedup 1.0000x reference)
"""MultiHeadSelfAttention Trainium2 kernel, head-sharded across 8 NeuronCores.

Reference computation (B=2, T=2048, D=1024, H=16, HS=64):
    k = einsum('btd,hdk->bhtk', x, Wk); q, v likewise
    wei = softmax(causal_mask(k @ q^T / sqrt(HS)), axis=-1)    # k in query role
    out = (wei @ v) concat-heads @ Wp + bp

Sharding: 2 heads per core (tensor parallel). Everything on-chip is computed
in "transposed" layout [feature, token] so the contraction dim of every matmul
sits on SBUF partitions. Per batch, the pipeline is projections -> V transpose
-> streaming causal attention; batches overlap (batch-1 projections run under
batch-0 attention, which is ACT/exp-bound). The softmax denominator comes from
a ones-column appended to the V tiles inside the PV matmul; normalization
broadcasts the reciprocal row with a single selection-matrix matmul. Head
outputs are exchanged with one AllToAll per batch so each core projects its
own 256-token slice (contraction over all 1024 concat-features). Matmuls run
in float32r (~11-bit mantissa, full PE rate at free-dim >= 256).
"""
import numpy as np

import concourse.bacc as bacc
import concourse.tile as tile
import concourse.mybir as mybir
from concourse import bass_utils

B, T, D = 2, 2048, 1024
H, HS = 16, 64
FEAT = H * HS
N_CORES = 8
NTOK = B * T               # 4096
TCH = 512                  # token chunk (free dim of most matmuls)
ACH = TCH // 2             # per-batch a2a shard width (256 tokens)
NJ = T // TCH              # 4 t-chunks per batch
NG = NTOK // 128           # 32 global s-chunks
NDC = D // 128             # 8 d chunks
F32 = mybir.dt.float32
F32R = mybir.dt.float32r
BF16 = mybir.dt.bfloat16
PROJ_BF16 = True           # x / Wq / Wk / Wv in bf16 (halves proj DMA)
A2A_BF16 = True            # head-output exchange + final projection in bf16
EXPF = mybir.ActivationFunctionType.Exp

_CACHE = {}
TRACE = False
LAST_RESULTS = None


def _build(reps=1, no_collective=False, fake_cc=False):
    nc = bacc.Bacc("TRN2", target_bir_lowering=False, debug=False,
                   num_devices=1 if no_collective else N_CORES)

    DX = BF16 if PROJ_BF16 else F32
    DA = BF16 if A2A_BF16 else F32
    xT_d = nc.dram_tensor("xT", [D, NTOK], DX, kind="ExternalInput").ap()
    wq_d = nc.dram_tensor("wq", [D, 128], DX, kind="ExternalInput").ap()
    wk_d = nc.dram_tensor("wk", [D, 128], DX, kind="ExternalInput").ap()
    wv_d = nc.dram_tensor("wv", [D, 128], DX, kind="ExternalInput").ap()
    wp_d = nc.dram_tensor("wp", [FEAT, FEAT], DA, kind="ExternalInput").ap()
    bp_d = nc.dram_tensor("bpr", [128, 8], F32, kind="ExternalInput").ap()
    out_d = nc.dram_tensor("out", [FEAT, TCH], F32, kind="ExternalOutput").ap()

    ident_h = nc.inline_tensor(np.eye(128, dtype=np.float32), name="identc")
    ones_h = nc.inline_tensor(np.ones((128, 64), dtype=np.float32), name="onesc")
    mask_h = nc.inline_tensor(
        np.triu(np.ones((128, 128), dtype=np.float32)), name="maskc")
    esel_np = np.zeros((128, 128), dtype=np.float32)
    esel_np[64, 0:64] = 1.0
    esel_np[96, 64:128] = 1.0
    esel_h = nc.inline_tensor(esel_np, name="eselc")
    zeros_h = nc.inline_tensor(np.zeros((128, TCH), dtype=np.float32),
                               name="zeroc")

    with tile.TileContext(nc) as tc:
        with (
            tc.tile_pool(name="const", bufs=1) as cpool,
            tc.tile_pool(name="persist", bufs=1) as ppool,
            tc.tile_pool(name="xs", bufs=3) as xpool,
            tc.tile_pool(name="wei", bufs=4) as wpool,
            tc.tile_pool(name="small", bufs=2) as spool,
            tc.tile_pool(name="outs", bufs=3) as opool,
            tc.tile_pool(name="gin", bufs=2) as gpool,
            tc.tile_pool(name="dram", bufs=1, space="DRAM") as dpool,
            tc.tile_pool(name="psp", bufs=2, space="PSUM") as pspool,
            tc.tile_pool(name="pss", bufs=2, space="PSUM") as sspool,
            tc.tile_pool(name="pso", bufs=1, space="PSUM") as sopool,
        ):
            # ---- constants ----
            wq_sb = cpool.tile([128, NDC * 128], DX)
            wk_sb = cpool.tile([128, NDC * 128], DX)
            wv_sb = cpool.tile([128, NDC * 128], DX)
            ident = cpool.tile([128, 128], F32R)
            mask = cpool.tile([128, 128], F32R)
            esel = cpool.tile([128, 128], F32R)
            bp_sb = cpool.tile([128, 8], F32)
            wp_sb = cpool.tile([128, NDC * FEAT], DA)
            for w_sb, w_d in ((wq_sb, wq_d), (wk_sb, wk_d), (wv_sb, wv_d)):
                nc.sync.dma_start(
                    w_sb[:].rearrange("p (c f) -> p c f", c=NDC),
                    w_d.rearrange("(c p) f -> p c f", p=128))
            nc.sync.dma_start(ident[:], ident_h.ap().bitcast(F32R))
            nc.sync.dma_start(mask[:], mask_h.ap().bitcast(F32R))
            nc.sync.dma_start(esel[:], esel_h.ap().bitcast(F32R))

            # ---- persistent activations (split per batch: avoids any
            # whole-tile WAR between batch-1 writers and batch-0 readers) ----
            Qts, Kts, Vts, Vss = [], [], [], []
            for b in range(B):
                Qt_b = ppool.tile([128, T], F32R, tag=f"Qt{b}")
                Kt_b = ppool.tile([128, T], F32R, tag=f"Kt{b}")
                Vt_b = ppool.tile([128, T], F32R, tag=f"Vt{b}")
                Vs_b = ppool.tile([128, NG // B * 130], F32R, tag=f"Vs{b}")
                nc.sync.dma_start(
                    Vs_b.rearrange("p (g c) -> p g c", c=65)[:, :, 64:65],
                    ones_h.ap()[:, 0:2 * (NG // B)].bitcast(F32R))
                Qts.append(Qt_b)
                Kts.append(Kt_b)
                Vts.append(Vt_b)
                Vss.append(Vs_b)
            recs = []
            for par in range(2):        # rows 64/96 hold 1/l; rest stay 0
                rec_p = ppool.tile([128, TCH], F32R, tag=f"rec{par}")
                nc.sync.dma_start(rec_p[:], zeros_h.ap().bitcast(F32R))
                recs.append(rec_p)

            for _rep in range(reps):
                a2a_ins, a2a_outs = [], []
                for b in range(B):
                    a2a_i = dpool.tile([N_CORES * 128, ACH], DA,
                                       tag=f"a2ai{b}")
                    a2a_o = dpool.tile([N_CORES * 128, ACH], DA,
                                       tag=f"a2ao{b}")
                    a2a_ins.append(a2a_i)
                    a2a_outs.append(a2a_o)
                def proj(b, tj0, tj1):
                    Qt, Kt, Vt, Vs = Qts[b], Kts[b], Vts[b], Vss[b]
                    for tj in range(tj0, tj1):
                        t = NJ * b + tj
                        xt = xpool.tile([128, NDC * TCH], DX, tag="xt")
                        nc.sync.dma_start(
                            xt[:].rearrange("p (c f) -> p c f", c=NDC),
                            xT_d.rearrange("(c p) f -> p c f",
                                           p=128)[:, :,
                                                  TCH * t:TCH * (t + 1)])
                        c0 = TCH * tj
                        for w_sb, dst in ((wq_sb, Qt), (wk_sb, Kt),
                                          (wv_sb, Vt)):
                            pp = pspool.tile([128, TCH], F32, tag="pp")
                            for dc in range(NDC):
                                nc.tensor.matmul(
                                    pp[:], w_sb[:, 128 * dc:128 * (dc + 1)],
                                    xt[:, TCH * dc:TCH * (dc + 1)],
                                    start=dc == 0, stop=dc == NDC - 1)
                            nc.vector.tensor_copy(dst[:, c0:c0 + TCH], pp[:])
                        for g in range(4 * tj, 4 * tj + 4):
                            pst = pspool.tile([128, TCH], F32, tag="pp")
                            nc.tensor.transpose(
                                pst[:, 0:128].bitcast(F32R),
                                Vt[:, 128 * g:128 * (g + 1)], ident[:])
                            nc.vector.tensor_copy(
                                Vs[:, 130 * g:130 * g + 64], pst[:, 0:64])
                            nc.vector.tensor_copy(
                                Vs[:, 130 * g + 65:130 * g + 129],
                                pst[:, 64:128])

                def attn(b, j0, j1):
                    Qt, Kt, Vs = Qts[b], Kts[b], Vss[b]
                    for j in range(j0, j1):
                        to = TCH * j
                        psO = sopool.tile([128, 2 * TCH], F32, tag="psO")
                        nsc = 4 * j + 4
                        for i in range(nsc):
                            g = i
                            so = 128 * i
                            r = i - 4 * j
                            vt0 = 128 * r if r > 0 else 0
                            psS = sspool.tile([128, 2 * TCH], F32, tag="psS")
                            nc.tensor.matmul(
                                psS[:, vt0:TCH], Qt[0:64, so:so + 128],
                                Kt[0:64, to + vt0:to + TCH],
                                start=True, stop=True)
                            nc.tensor.matmul(
                                psS[:, TCH + vt0:2 * TCH],
                                Qt[64:128, so:so + 128],
                                Kt[64:128, to + vt0:to + TCH],
                                start=True, stop=True, tile_position=(64, 0))
                            wei = wpool.tile([128, 2 * TCH], F32R, tag="wei")
                            nc.scalar.activation(
                                wei.rearrange("p (h t) -> p h t",
                                              h=2)[:, :, vt0:TCH],
                                psS.rearrange("p (h t) -> p h t",
                                              h=2)[:, :, vt0:TCH],
                                EXPF, scale=0.125)
                            if r >= 0:  # partial sub-block on the diagonal
                                d0 = 128 * r
                                nc.vector.tensor_tensor(
                                    wei[:, d0:d0 + 128], wei[:, d0:d0 + 128],
                                    mask[:], op=mybir.AluOpType.mult)
                                nc.vector.tensor_tensor(
                                    wei[:, TCH + d0:TCH + d0 + 128],
                                    wei[:, TCH + d0:TCH + d0 + 128],
                                    mask[:], op=mybir.AluOpType.mult)
                            st, sp = i == 0, i == nsc - 1
                            nc.tensor.matmul(
                                psO[0:65, vt0:TCH],
                                Vs[:, 130 * g:130 * g + 65],
                                wei[:, vt0:TCH], start=st, stop=sp)
                            nc.tensor.matmul(
                                psO[0:65, TCH + vt0:2 * TCH],
                                Vs[:, 130 * g + 65:130 * g + 130],
                                wei[:, TCH + vt0:2 * TCH], start=st, stop=sp)

                        # softmax normalization; rows 64 hold l = sum(exp)
                        rec = recs[j % 2]
                        with nc.allow_low_precision(reason="softmax denom"):
                            nc.vector.reciprocal(rec[64:65, :],
                                                 psO[64:65, 0:TCH])
                            nc.vector.reciprocal(rec[96:97, :],
                                                 psO[64:65, TCH:2 * TCH])
                        psr = sspool.tile([128, 2 * TCH], F32, tag="psS")
                        nc.tensor.matmul(psr[:, 0:TCH], esel[:], rec[:],
                                         start=True, stop=True)
                        rb = spool.tile([128, TCH], F32, tag="rb")
                        nc.vector.tensor_copy(rb[:], psr[:, 0:TCH])
                        outT = opool.tile([128, TCH], DA, tag="outT")
                        nc.vector.tensor_tensor(outT[0:64, :],
                                                psO[0:64, 0:TCH],
                                                rb[0:64, :],
                                                op=mybir.AluOpType.mult)
                        nc.vector.tensor_tensor(outT[64:128, :],
                                                psO[0:64, TCH:2 * TCH],
                                                rb[64:128, :],
                                                op=mybir.AluOpType.mult)
                        nc.sync.dma_start(
                            a2a_ins[b][256 * j:256 * (j + 1),
                                       :].rearrange("(h p) t -> p h t", p=128),
                            outT[:].rearrange("p (h t) -> p h t", h=2))

                def a2a(b):
                    if no_collective or fake_cc:
                        nc.sync.dma_start(a2a_outs[b][:], a2a_ins[b][:])
                    else:
                        nc.gpsimd.collective_compute(
                            "AllToAll", mybir.AluOpType.bypass,
                            replica_groups=[list(range(N_CORES))],
                            ins=[a2a_ins[b][:].opt()],
                            outs=[a2a_outs[b][:].opt()])

                def final(b):
                    gin = gpool.tile([128, NDC * ACH], DA, tag="gin")
                    nc.sync.dma_start(
                        gin[:].rearrange("p (c f) -> p c f", c=NDC),
                        a2a_outs[b][:].rearrange("(c p) t -> p c t", p=128))
                    fstage = gpool.tile([128, NDC * ACH], F32, tag="fstage")
                    for ofc in range(NDC):
                        psf = pspool.tile([128, TCH], F32, tag="pp")
                        for ifc in range(NDC):
                            c0 = FEAT * ifc + 128 * ofc
                            nc.tensor.matmul(psf[:, 0:ACH],
                                             wp_sb[:, c0:c0 + 128],
                                             gin[:, ACH * ifc:ACH * (ifc + 1)],
                                             start=ifc == 0,
                                             stop=ifc == NDC - 1)
                        nc.vector.tensor_scalar_add(
                            fstage[:, ACH * ofc:ACH * (ofc + 1)],
                            psf[:, 0:ACH], bp_sb[:, ofc:ofc + 1])
                    nc.sync.dma_start(
                        out_d.rearrange("(c p) t -> p c t",
                                        p=128)[:, :, ACH * b:ACH * (b + 1)],
                        fstage[:].rearrange("p (c f) -> p c f", c=NDC))

                # software pipeline across the two batches
                proj(0, 0, NJ)
                attn(0, 0, 1)
                proj(1, 0, 1)
                attn(0, 1, 2)
                proj(1, 1, 2)
                attn(0, 2, 3)
                proj(1, 2, 3)
                attn(0, 3, 4)
                proj(1, 3, 4)
                a2a(0)
                if _rep == 0:
                    nc.sync.dma_start(bp_sb[:], bp_d[:])
                    nc.sync.dma_start(
                        wp_sb[:].rearrange("p (c f) -> p c f", c=NDC),
                        wp_d.rearrange("(c p) f -> p c f", p=128))
                attn(1, 0, 3)
                final(0)
                attn(1, 3, NJ)
                a2a(1)
                final(1)

    nc.compile()
    return nc


def make_in_maps(x, Wk, Wq, Wv, Wp, bp):
    import ml_dtypes
    dx = ml_dtypes.bfloat16 if PROJ_BF16 else np.float32
    da = ml_dtypes.bfloat16 if A2A_BF16 else np.float32
    x = np.ascontiguousarray(np.asarray(x, dtype=np.float32))
    Wk = np.asarray(Wk, dtype=np.float32).astype(dx)
    Wq = np.asarray(Wq, dtype=np.float32).astype(dx)
    Wv = np.asarray(Wv, dtype=np.float32).astype(dx)
    Wp = np.ascontiguousarray(np.asarray(Wp, dtype=np.float32).astype(da))
    bp = np.asarray(bp, dtype=np.float32)
    xT = np.ascontiguousarray(x.reshape(NTOK, D).T.astype(dx))
    bpr = np.ascontiguousarray(bp.reshape(8, 128).T)
    in_maps = []
    for c in range(N_CORES):
        hA, hB = 2 * c, 2 * c + 1
        in_maps.append({
            "xT": xT,
            "wq": np.ascontiguousarray(
                np.concatenate([Wq[hA], Wq[hB]], axis=1)),
            "wk": np.ascontiguousarray(
                np.concatenate([Wk[hA], Wk[hB]], axis=1)),
            "wv": np.ascontiguousarray(
                np.concatenate([Wv[hA], Wv[hB]], axis=1)),
            "wp": Wp,
            "bpr": bpr,
        })
    return in_maps


def kernel(x, Wk, Wq, Wv, Wp, bp):
    global LAST_RESULTS
    in_maps = make_in_maps(x, Wk, Wq, Wv, Wp, bp)
    if "nc" not in _CACHE:
        _CACHE["nc"] = _build()
    nc = _CACHE["nc"]

    res = bass_utils.run_bass_kernel_spmd(
        nc, in_maps, core_ids=list(range(N_CORES)), trace=TRACE)
    LAST_RESULTS = res

    out = np.empty((NTOK, FEAT), dtype=np.float32)
    for c in range(N_CORES):
        o = res.results[c]["out"]
        out[ACH * c:ACH * (c + 1), :] = o[:, 0:ACH].T
        out[T + ACH * c:T + ACH * (c + 1), :] = o[:, ACH:TCH].T
    return out.reshape(B, T, FEAT)
